# revision 10
# baseline (speedup 1.0000x reference)
"""BinaryLinear 2:4 kernel for trn2 (8 NeuronCores).

Computes: out = binarize(weight * mask_2_4(weight)) @ x
  - mask_2_4: keep 2 largest-|.| of every 4 along the reduction dim
    (ties broken toward lower index, matching jax.lax.top_k)
  - binarize: kept positive -> 1.0, else 0.0
  - out = wb @ x, (4096, 4096) fp32

Strategy: shard weight rows (outfeatures) 8 ways; replicate x; each core
computes its (512, 4096) output slice; host concatenates.

Per-core pipeline:
  Phase A (mask): for each 128-row weight block, compute wb in {0,1} fp16
    via exact fp32 pairwise comparisons on the vector engine, store to a
    DRAM bounce buffer, then DMA-transpose into k-major lhsT layout.
  Phase B (matmul): stream x in column chunks, cast fp32->fp16 on the
    scalar engine, accumulate fp16 matmuls (exact {0,1} weights) in fp32
    PSUM over the full K=4096, copy out.
"""

import numpy as np

# Full problem shapes (hardcoded per contract).
M_FULL = 4096  # outfeatures
K_FULL = 4096  # infeatures (reduction; 2:4 groups along this dim)
N_FULL = 4096  # ncols of x
N_CORES = 8
M_SHARD = M_FULL // N_CORES  # 512 weight rows per core

_CACHE = {}


def _build_bass(M=M_SHARD, K=K_FULL, N=N_FULL, k_sub=1024, n_chunk=256):
    import concourse.bass as bass
    import concourse.tile as tile
    from concourse import bacc, mybir
    from contextlib import ExitStack

    dt = mybir.dt
    f32 = dt.float32
    f16 = dt.float16
    bf16 = dt.bfloat16
    Alu = mybir.AluOpType
    Act = mybir.ActivationFunctionType

    P = 128
    MB = M // P            # m-blocks per core
    KS = K // k_sub        # k sub-blocks for phase A
    KT = K // P            # k tiles (contraction) for matmul
    NCH = N // n_chunk     # x column chunks
    Q = k_sub // 4         # quarter size within a k sub-block
    TPS = k_sub // P       # transposes per k sub-block

    nc = bacc.Bacc()
    w_d = nc.declare_dram_parameter("w", [M, K], f32, isOutput=False)
    x_d = nc.declare_dram_parameter("x", [K, N], f32, isOutput=False)
    out_d = nc.declare_dram_parameter("out", [M, N], f32, isOutput=True)

    with tile.TileContext(nc) as tc:
        with ExitStack() as ctx:
            wbt_pool = ctx.enter_context(tc.tile_pool(name="wbt", bufs=1))
            pa = ctx.enter_context(tc.tile_pool(name="pa", bufs=2))
            xs = ctx.enter_context(tc.tile_pool(name="xs", bufs=2))
            ps_pool = ctx.enter_context(tc.tile_pool(name="ps", bufs=4, space="PSUM"))
            op_pool = ctx.enter_context(tc.tile_pool(name="op", bufs=3))

            # lhsT layout: [k_in(128 partitions), k_tile, m]
            wbt = wbt_pool.tile([P, KT, M], f16)

            # ---------------- Phase A: 2:4 mask + binarize ----------------
            for ks in range(KS):
                for mb in range(MB):
                    wsub = pa.tile([P, k_sub], f32, tag="wsub")
                    nc.sync.dma_start(
                        wsub[:], w_d[mb * P:(mb + 1) * P, ks * k_sub:(ks + 1) * k_sub]
                    )
                    w4 = wsub.rearrange("p (g j) -> p g j", j=4)  # (P, Q, 4)

                    # |w| per group lane, deinterleaved to contiguous quarters.
                    # Exact abs = clear the sign bit (bitwise AND on int32
                    # bitcast views; walrus rejects abs_max on DVE).
                    i32 = dt.int32
                    w4i = wsub[:].bitcast(i32).rearrange("p (g j) -> p g j", j=4)
                    a = pa.tile([P, 4, Q], f32, tag="absq")
                    for j in range(4):
                        nc.vector.tensor_scalar(
                            a[:, j].bitcast(i32), w4i[:, :, j],
                            0x7FFFFFFF, None, Alu.bitwise_and,
                        )

                    # Pairwise comparisons (exact fp32 inputs; {0,1} bf16 out).
                    # gij = [a_i >= a_j]  (i<j: i wins ties, matching top_k)
                    # h03 = [a0 < a3]
                    def cmp(name, i, j, op):
                        t = pa.tile([P, Q], bf16, tag=name)
                        nc.vector.tensor_tensor(t[:], a[:, i], a[:, j], op)
                        return t

                    g01 = cmp("g01", 0, 1, Alu.is_ge)
                    g02 = cmp("g02", 0, 2, Alu.is_ge)
                    h03 = cmp("h03", 0, 3, Alu.is_lt)
                    g12 = cmp("g12", 1, 2, Alu.is_ge)
                    g13 = cmp("g13", 1, 3, Alu.is_ge)
                    g23 = cmp("g23", 2, 3, Alu.is_ge)

                    # keep_i: rank_i <= 1  (small-int bf16 arithmetic, exact)
                    #   keep0 = (g01 + g02) - 1 >= h03
                    #   keep1 = (g12 + g13) - 1 >= g01
                    #   keep2 = (g02 + g12)     <= g23
                    #   keep3 = (g13 + g23)     <= h03
                    def keep(name, u, v, bias, opc, other):
                        t = pa.tile([P, Q], bf16, tag=name + "s")
                        nc.vector.tensor_tensor(t[:], u[:], v[:], Alu.add)
                        if bias:
                            nc.vector.tensor_scalar(t[:], t[:], bias, None, Alu.add)
                        k = pa.tile([P, Q], bf16, tag=name)
                        nc.vector.tensor_tensor(k[:], t[:], other[:], opc)
                        return k

                    k0 = keep("k0", g01, g02, -1.0, Alu.is_ge, h03)
                    k1 = keep("k1", g12, g13, -1.0, Alu.is_ge, g01)
                    k2 = keep("k2", g02, g12, 0.0, Alu.is_le, g23)
                    k3 = keep("k3", g13, g23, 0.0, Alu.is_le, h03)

                    # wb_j = [w_j > 0] * keep_j  -> fp16 {0,1}, interleaved
                    # (pos on gpsimd so wsub has a single reader engine)
                    wb = pa.tile([P, k_sub], f16, tag="wbq")
                    wb4 = wb.rearrange("p (g j) -> p g j", j=4)
                    pos = pa.tile([P, 4, Q], bf16, tag="posq")
                    for j in range(4):
                        nc.vector.tensor_scalar(
                            pos[:, j], w4[:, :, j], 0.0, None, Alu.is_gt
                        )
                    for j, kj in enumerate((k0, k1, k2, k3)):
                        nc.gpsimd.tensor_tensor(
                            wb4[:, :, j], pos[:, j], kj[:], Alu.mult
                        )

                    # Transpose into lhsT layout (SBUF -> SBUF, 128x128 tiles)
                    for t in range(TPS):
                        kt = ks * TPS + t
                        nc.sync.dma_start_transpose(
                            wbt[:, kt, mb * P:(mb + 1) * P],
                            wb[:, t * P:(t + 1) * P],
                        )

            # ---------------- Phase B: streamed matmul ----------------
            x_r = x_d.rearrange("(ko p) n -> p ko n", p=P)  # (P, KT, N)
            for nch in range(NCH):
                nsl = slice(nch * n_chunk, (nch + 1) * n_chunk)
                xf = xs.tile([P, KT, n_chunk], f32, tag="xf")
                nc.sync.dma_start(xf[:], x_r[:, :, nsl])
                xh = xs.tile([P, KT, n_chunk], f16, tag="xh")
                nc.scalar.activation(xh[:], xf[:], Act.Copy)

                for mb in range(MB):
                    ps = ps_pool.tile([P, n_chunk], f32, tag="ps")
                    for kt in range(KT):
                        nc.tensor.matmul(
                            ps[:],
                            lhsT=wbt[:, kt, mb * P:(mb + 1) * P],
                            rhs=xh[:, kt, :],
                            start=(kt == 0),
                            stop=(kt == KT - 1),
                        )
                    ob = op_pool.tile([P, n_chunk], f32, tag="ob")
                    nc.vector.tensor_copy(ob[:], ps[:])
                    nc.sync.dma_start(out_d[mb * P:(mb + 1) * P, nsl], ob[:])

    nc.finalize()
    return nc


def _get_nc():
    if "nc" not in _CACHE:
        _CACHE["nc"] = _build_bass()
    return _CACHE["nc"]


def kernel(x: np.ndarray, weight: np.ndarray) -> np.ndarray:
    from concourse.bass_utils import run_bass_kernel_spmd

    x = np.ascontiguousarray(np.asarray(x, dtype=np.float32))
    weight = np.ascontiguousarray(np.asarray(weight, dtype=np.float32))
    assert x.shape == (K_FULL, N_FULL) and weight.shape == (M_FULL, K_FULL)

    nc = _get_nc()
    in_maps = [
        {
            "w": np.ascontiguousarray(weight[c * M_SHARD:(c + 1) * M_SHARD, :]),
            "x": x,
        }
        for c in range(N_CORES)
    ]
    res = run_bass_kernel_spmd(nc, in_maps, list(range(N_CORES)))
    out = np.concatenate(
        [res.results[c]["out"] for c in range(N_CORES)], axis=0
    ).astype(np.float32)
    return out


# revision 15
# speedup vs baseline: 1.4819x; 1.4819x over previous
"""BinaryLinear 2:4 kernel for trn2 (8 NeuronCores).

Computes: out = binarize(weight * mask_2_4(weight)) @ x
  - mask_2_4: keep 2 largest-|.| of every 4 along the reduction dim
    (ties broken toward lower index, matching jax.lax.top_k)
  - binarize: kept positive -> 1.0, else 0.0
  - out = wb @ x, (4096, 4096) fp32

Strategy: shard weight rows (outfeatures) 8 ways; replicate x; each core
computes its (512, 4096) output slice; host concatenates.

Per-core pipeline:
  Phase A (mask): for each 128-row weight block, compute wb in {0,1} fp16
    via exact fp32 pairwise comparisons on the vector engine, store to a
    DRAM bounce buffer, then DMA-transpose into k-major lhsT layout.
  Phase B (matmul): stream x in column chunks, cast fp32->fp16 on the
    scalar engine, accumulate fp16 matmuls (exact {0,1} weights) in fp32
    PSUM over the full K=4096, copy out.
"""

import numpy as np

# Full problem shapes (hardcoded per contract).
M_FULL = 4096  # outfeatures
K_FULL = 4096  # infeatures (reduction; 2:4 groups along this dim)
N_FULL = 4096  # ncols of x
N_CORES = 8
M_SHARD = M_FULL // N_CORES  # 512 weight rows per core

_CACHE = {}


def _build_bass(M=M_SHARD, K=K_FULL, N=N_FULL, k_sub=1024, n_chunk=256):
    import concourse.bass as bass
    import concourse.tile as tile
    from concourse import bacc, mybir
    from contextlib import ExitStack

    dt = mybir.dt
    f32 = dt.float32
    f16 = dt.float16
    bf16 = dt.bfloat16
    Alu = mybir.AluOpType
    Act = mybir.ActivationFunctionType

    P = 128
    MB = M // P            # m-blocks per core
    KS = K // k_sub        # k sub-blocks for phase A
    KT = K // P            # k tiles (contraction) for matmul
    NCH = N // n_chunk     # x column chunks
    Q = k_sub // 4         # quarter size within a k sub-block
    TPS = k_sub // P       # transposes per k sub-block

    nc = bacc.Bacc()
    w_d = nc.declare_dram_parameter("w", [M, K], f32, isOutput=False)
    x_d = nc.declare_dram_parameter("x", [K, N], f32, isOutput=False)
    out_d = nc.declare_dram_parameter("out", [M, N], f32, isOutput=True)

    with tile.TileContext(nc) as tc:
        with ExitStack() as ctx:
            wbt_pool = ctx.enter_context(tc.tile_pool(name="wbt", bufs=1))
            pa = ctx.enter_context(tc.tile_pool(name="pa", bufs=2))
            xs_f = ctx.enter_context(tc.tile_pool(name="xsf", bufs=4))
            xs = ctx.enter_context(tc.tile_pool(name="xs", bufs=3))
            ps_pool = ctx.enter_context(tc.tile_pool(name="ps", bufs=6, space="PSUM"))
            op_pool = ctx.enter_context(tc.tile_pool(name="op", bufs=4))

            # lhsT layout: [k_in(128 partitions), k_tile, m]
            wbt = wbt_pool.tile([P, KT, M], f16)

            # ---------------- Phase A: 2:4 mask + binarize ----------------
            # mb outer so each m-block's lhsT completes early and phase B
            # overlaps the rest of phase A.
            for mb in range(MB):
                wb_row = pa.tile([P, K], f16, tag="wbrow")
                for ks in range(KS):
                    wsub = pa.tile([P, k_sub], f32, tag="wsub")
                    nc.gpsimd.dma_start(
                        wsub[:], w_d[mb * P:(mb + 1) * P, ks * k_sub:(ks + 1) * k_sub]
                    )
                    w4 = wsub.rearrange("p (g j) -> p g j", j=4)  # (P, Q, 4)

                    # |w| per group lane, deinterleaved to contiguous quarters.
                    # Exact abs = clear the sign bit (bitwise AND on int32
                    # bitcast views; walrus rejects abs_max on DVE).
                    i32 = dt.int32
                    w4i = wsub[:].bitcast(i32).rearrange("p (g j) -> p g j", j=4)
                    a = pa.tile([P, 4, Q], f32, tag="absq")
                    for j in range(4):
                        nc.vector.tensor_scalar(
                            a[:, j].bitcast(i32), w4i[:, :, j],
                            0x7FFFFFFF, None, Alu.bitwise_and,
                        )

                    # Pairwise comparisons (exact fp32 inputs; {0,1} bf16 out).
                    # gij = [a_i >= a_j]  (i<j: i wins ties, matching top_k)
                    # h03 = [a0 < a3]
                    def cmp(name, i, j, op):
                        t = pa.tile([P, Q], bf16, tag=name)
                        nc.vector.tensor_tensor(t[:], a[:, i], a[:, j], op)
                        return t

                    g01 = cmp("g01", 0, 1, Alu.is_ge)
                    g02 = cmp("g02", 0, 2, Alu.is_ge)
                    h03 = cmp("h03", 0, 3, Alu.is_lt)
                    g12 = cmp("g12", 1, 2, Alu.is_ge)
                    g13 = cmp("g13", 1, 3, Alu.is_ge)
                    g23 = cmp("g23", 2, 3, Alu.is_ge)

                    # keep_i: rank_i <= 1  (small-int bf16 arithmetic, exact)
                    #   keep0 = (g01 + g02) - 1 >= h03
                    #   keep1 = (g12 + g13) - 1 >= g01
                    #   keep2 = (g02 + g12)     <= g23
                    #   keep3 = (g13 + g23)     <= h03
                    def keep(name, u, v, bias, opc, other):
                        t = pa.tile([P, Q], bf16, tag=name + "s")
                        nc.vector.tensor_tensor(t[:], u[:], v[:], Alu.add)
                        if bias:
                            nc.vector.tensor_scalar(t[:], t[:], bias, None, Alu.add)
                        k = pa.tile([P, Q], bf16, tag=name)
                        nc.vector.tensor_tensor(k[:], t[:], other[:], opc)
                        return k

                    k0 = keep("k0", g01, g02, -1.0, Alu.is_ge, h03)
                    k1 = keep("k1", g12, g13, -1.0, Alu.is_ge, g01)
                    k2 = keep("k2", g02, g12, 0.0, Alu.is_le, g23)
                    k3 = keep("k3", g13, g23, 0.0, Alu.is_le, h03)

                    # wb_j = [w_j > 0] * keep_j  -> fp16 {0,1}, interleaved
                    wb4 = wb_row[:, ks * k_sub:(ks + 1) * k_sub].rearrange(
                        "p (g j) -> p g j", j=4
                    )
                    pos = pa.tile([P, 4, Q], bf16, tag="posq")
                    for j in range(4):
                        nc.vector.tensor_scalar(
                            pos[:, j], w4[:, :, j], 0.0, None, Alu.is_gt
                        )
                    for j, kj in enumerate((k0, k1, k2, k3)):
                        nc.gpsimd.tensor_tensor(
                            wb4[:, :, j], pos[:, j], kj[:], Alu.mult
                        )

                # One SBUF->SBUF xbar transpose per m-block into lhsT layout:
                # out[kp, kt, m] = wb_row[m, kt*128 + kp] (middle-major 3D).
                nc.sync.dma_start_transpose(
                    wbt[:, :, mb * P:(mb + 1) * P], wb_row[:]
                )

            # ---------------- Phase B: streamed matmul ----------------
            x_r = x_d.rearrange("(ko p) n -> p ko n", p=P)  # (P, KT, N)
            XP = min(8, KT)  # ko-granularity of x load/cast pieces
            for nch in range(NCH):
                nsl = slice(nch * n_chunk, (nch + 1) * n_chunk)
                xh = xs.tile([P, KT, n_chunk], f16, tag="xh")
                for pc in range(KT // XP):
                    xf = xs_f.tile([P, XP, n_chunk], f32, tag="xf")
                    nc.sync.dma_start(
                        xf[:], x_r[:, pc * XP:(pc + 1) * XP, nsl]
                    )
                    nc.scalar.activation(
                        xh[:, pc * XP:(pc + 1) * XP, :], xf[:], Act.Copy
                    )

                for mb in range(MB):
                    ps = ps_pool.tile([P, n_chunk], f32, tag="ps")
                    for kt in range(KT):
                        nc.tensor.matmul(
                            ps[:],
                            lhsT=wbt[:, kt, mb * P:(mb + 1) * P],
                            rhs=xh[:, kt, :],
                            start=(kt == 0),
                            stop=(kt == KT - 1),
                        )
                    ob = op_pool.tile([P, n_chunk], f32, tag="ob")
                    nc.vector.tensor_copy(ob[:], ps[:])
                    nc.gpsimd.dma_start(out_d[mb * P:(mb + 1) * P, nsl], ob[:])

    nc.finalize()
    return nc


def _get_nc():
    if "nc" not in _CACHE:
        _CACHE["nc"] = _build_bass()
    return _CACHE["nc"]


def kernel(x: np.ndarray, weight: np.ndarray) -> np.ndarray:
    from concourse.bass_utils import run_bass_kernel_spmd

    x = np.ascontiguousarray(np.asarray(x, dtype=np.float32))
    weight = np.ascontiguousarray(np.asarray(weight, dtype=np.float32))
    assert x.shape == (K_FULL, N_FULL) and weight.shape == (M_FULL, K_FULL)

    nc = _get_nc()
    in_maps = [
        {
            "w": np.ascontiguousarray(weight[c * M_SHARD:(c + 1) * M_SHARD, :]),
            "x": x,
        }
        for c in range(N_CORES)
    ]
    res = run_bass_kernel_spmd(nc, in_maps, list(range(N_CORES)))
    out = np.concatenate(
        [res.results[c]["out"] for c in range(N_CORES)], axis=0
    ).astype(np.float32)
    return out


# revision 17
# speedup vs baseline: 1.5596x; 1.0525x over previous
"""BinaryLinear 2:4 kernel for trn2 (8 NeuronCores).

Computes: out = binarize(weight * mask_2_4(weight)) @ x
  - mask_2_4: keep 2 largest-|.| of every 4 along the reduction dim
    (ties broken toward lower index, matching jax.lax.top_k)
  - binarize: kept positive -> 1.0, else 0.0
  - out = wb @ x, (4096, 4096) fp32

Strategy: shard weight rows (outfeatures) 8 ways; replicate x; each core
computes its (512, 4096) output slice; host concatenates.

Per-core pipeline:
  Phase A (mask): for each 128-row weight block, compute wb in {0,1} fp16
    via exact fp32 pairwise comparisons on the vector engine, store to a
    DRAM bounce buffer, then DMA-transpose into k-major lhsT layout.
  Phase B (matmul): stream x in column chunks, cast fp32->fp16 on the
    scalar engine, accumulate fp16 matmuls (exact {0,1} weights) in fp32
    PSUM over the full K=4096, copy out.
"""

import numpy as np

# Full problem shapes (hardcoded per contract).
M_FULL = 4096  # outfeatures
K_FULL = 4096  # infeatures (reduction; 2:4 groups along this dim)
N_FULL = 4096  # ncols of x
N_CORES = 8
M_SHARD = M_FULL // N_CORES  # 512 weight rows per core

_CACHE = {}


def _build_bass(M=M_SHARD, K=K_FULL, N=N_FULL, k_sub=1024, n_chunk=256):
    import concourse.bass as bass
    import concourse.tile as tile
    from concourse import bacc, mybir
    from contextlib import ExitStack

    dt = mybir.dt
    f32 = dt.float32
    f16 = dt.float16
    bf16 = dt.bfloat16
    Alu = mybir.AluOpType
    Act = mybir.ActivationFunctionType

    P = 128
    MB = M // P            # m-blocks per core
    KS = K // k_sub        # k sub-blocks for phase A
    KT = K // P            # k tiles (contraction) for matmul
    NCH = N // n_chunk     # x column chunks
    Q = k_sub // 4         # quarter size within a k sub-block
    TPS = k_sub // P       # transposes per k sub-block

    nc = bacc.Bacc()
    w_d = nc.declare_dram_parameter("w", [M, K], f32, isOutput=False)
    x_d = nc.declare_dram_parameter("x", [K, N], f32, isOutput=False)
    out_d = nc.declare_dram_parameter("out", [M, N], f32, isOutput=True)

    with tile.TileContext(nc) as tc:
        with ExitStack() as ctx:
            wbt_pool = ctx.enter_context(tc.tile_pool(name="wbt", bufs=1))
            pa = ctx.enter_context(tc.tile_pool(name="pa", bufs=2))
            xs_f = ctx.enter_context(tc.tile_pool(name="xsf", bufs=4))
            xs = ctx.enter_context(tc.tile_pool(name="xs", bufs=3))
            ps_pool = ctx.enter_context(tc.tile_pool(name="ps", bufs=6, space="PSUM"))
            op_pool = ctx.enter_context(tc.tile_pool(name="op", bufs=4))

            # lhsT layout: [k_in(128 partitions), k_tile, m]
            wbt = wbt_pool.tile([P, KT, M], f16)

            # ---------------- Phase A: 2:4 mask + binarize ----------------
            # mb outer so each m-block's lhsT completes early and phase B
            # overlaps the rest of phase A.
            i32 = dt.int32
            for mb in range(MB):
                for ks in range(KS):
                    wsub = pa.tile([P, k_sub], f32, tag="wsub")
                    nc.gpsimd.dma_start(
                        wsub[:], w_d[mb * P:(mb + 1) * P, ks * k_sub:(ks + 1) * k_sub]
                    )

                    # Exact |w| = clear the sign bit (bitwise AND on int32
                    # bitcast views; walrus rejects abs_max on DVE). One
                    # contiguous op; comparisons below use strided lanes.
                    aq = pa.tile([P, k_sub], f32, tag="absq")
                    nc.vector.tensor_scalar(
                        aq[:].bitcast(i32), wsub[:].bitcast(i32),
                        0x7FFFFFFF, None, Alu.bitwise_and,
                    )
                    # pos = [w > 0] (one contiguous op, interleaved layout)
                    pos = pa.tile([P, k_sub], bf16, tag="posq")
                    nc.vector.tensor_scalar(
                        pos[:], wsub[:], 0.0, None, Alu.is_gt
                    )

                    a4 = aq.rearrange("p (g j) -> p g j", j=4)  # (P, Q, 4)

                    # Pairwise comparisons (exact fp32 inputs; {0,1} bf16 out).
                    # gij = [a_i >= a_j]  (i<j: i wins ties, matching top_k)
                    # h03 = [a0 < a3]
                    def cmp(name, i, j, op):
                        t = pa.tile([P, Q], bf16, tag=name)
                        nc.vector.tensor_tensor(t[:], a4[:, :, i], a4[:, :, j], op)
                        return t

                    g01 = cmp("g01", 0, 1, Alu.is_ge)
                    g02 = cmp("g02", 0, 2, Alu.is_ge)
                    h03 = cmp("h03", 0, 3, Alu.is_lt)
                    g12 = cmp("g12", 1, 2, Alu.is_ge)
                    g13 = cmp("g13", 1, 3, Alu.is_ge)
                    g23 = cmp("g23", 2, 3, Alu.is_ge)

                    # keep_i: rank_i <= 1  (small-int bf16 arithmetic, exact)
                    #   keep0 = g01 + g02 > h03
                    #   keep1 = g12 + g13 > g01
                    #   keep2 = g02 + g12 <= g23
                    #   keep3 = g13 + g23 <= h03
                    def keep(name, u, v, opc, other):
                        t = pa.tile([P, Q], bf16, tag=name + "s")
                        nc.vector.tensor_tensor(t[:], u[:], v[:], Alu.add)
                        k = pa.tile([P, Q], bf16, tag=name)
                        nc.vector.tensor_tensor(k[:], t[:], other[:], opc)
                        return k

                    k0 = keep("k0", g01, g02, Alu.is_gt, h03)
                    k1 = keep("k1", g12, g13, Alu.is_gt, g01)
                    k2 = keep("k2", g02, g12, Alu.is_le, g23)
                    k3 = keep("k3", g13, g23, Alu.is_le, h03)

                    # wb_j = pos_j * keep_j  -> fp16 {0,1}, interleaved
                    wb = pa.tile([P, k_sub], f16, tag="wbq")
                    wb4 = wb.rearrange("p (g j) -> p g j", j=4)
                    pos4 = pos.rearrange("p (g j) -> p g j", j=4)
                    for j, kj in enumerate((k0, k1, k2, k3)):
                        nc.gpsimd.tensor_tensor(
                            wb4[:, :, j], pos4[:, :, j], kj[:], Alu.mult
                        )

                    # SBUF->SBUF xbar transpose into lhsT layout (3D out is
                    # middle-major: out[kp, c, m] = wb[m, c*128 + kp]).
                    nc.sync.dma_start_transpose(
                        wbt[:, ks * TPS:(ks + 1) * TPS, mb * P:(mb + 1) * P],
                        wb[:],
                    )

            # ---------------- Phase B: streamed matmul ----------------
            x_r = x_d.rearrange("(ko p) n -> p ko n", p=P)  # (P, KT, N)
            XP = min(8, KT)  # ko-granularity of x load/cast pieces
            for nch in range(NCH):
                nsl = slice(nch * n_chunk, (nch + 1) * n_chunk)
                xh = xs.tile([P, KT, n_chunk], f16, tag="xh")
                for pc in range(KT // XP):
                    xf = xs_f.tile([P, XP, n_chunk], f32, tag="xf")
                    nc.sync.dma_start(
                        xf[:], x_r[:, pc * XP:(pc + 1) * XP, nsl]
                    )
                    nc.scalar.activation(
                        xh[:, pc * XP:(pc + 1) * XP, :], xf[:], Act.Copy
                    )

                for mb in range(MB):
                    ps = ps_pool.tile([P, n_chunk], f32, tag="ps")
                    for kt in range(KT):
                        nc.tensor.matmul(
                            ps[:],
                            lhsT=wbt[:, kt, mb * P:(mb + 1) * P],
                            rhs=xh[:, kt, :],
                            start=(kt == 0),
                            stop=(kt == KT - 1),
                        )
                    ob = op_pool.tile([P, n_chunk], f32, tag="ob")
                    nc.scalar.activation(ob[:], ps[:], Act.Copy)
                    nc.gpsimd.dma_start(out_d[mb * P:(mb + 1) * P, nsl], ob[:])

    nc.finalize()
    return nc


def _get_nc():
    if "nc" not in _CACHE:
        _CACHE["nc"] = _build_bass()
    return _CACHE["nc"]


def kernel(x: np.ndarray, weight: np.ndarray) -> np.ndarray:
    from concourse.bass_utils import run_bass_kernel_spmd

    x = np.ascontiguousarray(np.asarray(x, dtype=np.float32))
    weight = np.ascontiguousarray(np.asarray(weight, dtype=np.float32))
    assert x.shape == (K_FULL, N_FULL) and weight.shape == (M_FULL, K_FULL)

    nc = _get_nc()
    in_maps = [
        {
            "w": np.ascontiguousarray(weight[c * M_SHARD:(c + 1) * M_SHARD, :]),
            "x": x,
        }
        for c in range(N_CORES)
    ]
    res = run_bass_kernel_spmd(nc, in_maps, list(range(N_CORES)))
    out = np.concatenate(
        [res.results[c]["out"] for c in range(N_CORES)], axis=0
    ).astype(np.float32)
    return out
